# revision 17
# baseline (speedup 1.0000x reference)
"""Trainium2 Bass kernel for a transformer decoder layer (self-attn + cross-attn + FFN).

Distribution over 8 NeuronCores:
  * self-attention: TP=8 over heads (2 heads/core) with BOTH batches
    concatenated on the token axis (4096 token-instances per core); the
    attention context is exchanged with a single 8-rank AllToAll (1MB) so each
    core ends up with all 1024 context features for its 512 tokens, then the
    full O-projection runs locally (no ReduceScatter).
  * cross-attention K/V: computed head-sharded per 4-core batch group from
    enc_out FIRST, and AllGathered under self-attention (off critical path).
    Cross-attention itself runs token-sharded with all 16 heads per core.
  * LayerNorms + FFN: token-sharded (512 tokens/core), no further comm.

All activations are feature-major ([D, seq]) on chip; matmuls are bf16 with
fp32 PSUM accumulation; softmax uses a constant exp-shift (exact for softmax)
with scores^T ([k, q]) layout.  The stationary [V | ones*64] trick makes each
attV matmul produce 64 broadcast copies of the softmax denominator, so
normalization is pure DVE (no PE broadcast matmuls).  LN statistics use an
all-ones [P,128] stationary so mean/var arrive pre-broadcast on 128
partitions.
"""

import sys

sys.path.insert(0, "/opt/trn_rl_repo")

import numpy as np
from ml_dtypes import bfloat16

import concourse.bass as bass
import concourse.mybir as mybir
import concourse.tile as tile
from concourse.tile_rust import add_dep_helper
from concourse import bacc
from concourse import bass_utils

F32 = mybir.dt.float32
BF16 = mybir.dt.bfloat16
ALU = mybir.AluOpType
AF = mybir.ActivationFunctionType

P = 128
EPS = 1e-5
EXP_SHIFT = -12.0  # exp(s + EXP_SHIFT): overflow headroom, exact for softmax
NEG = -1e9

SKIP = -2
FULL = -1

ECH = 4  # k-blocks per attention-weight tile


class Cfg:
    def __init__(self, S, D, H, FF, TP, B):
        self.S, self.D, self.H, self.FF, self.B = S, D, H, FF, B
        self.G = TP                  # cross-attn group size (per batch)
        self.DK = 64
        self.n_cores = TP * B        # 8
        self.TQ = B * S              # self-attn token instances
        self.HLs = H // self.n_cores # self local heads (2)
        self.MHs = self.HLs * self.DK
        self.NQ = self.TQ // 512     # self q blocks (8)
        self.NKs = self.TQ // P      # self k blocks (32)
        self.HLc = H // TP           # cross local heads for the AG (4)
        self.MHc = self.HLc * self.DK
        self.MOc = self.MHc // P
        self.NKc = S // P            # cross k blocks (16)
        self.SB = S // TP            # token chunk (512)
        self.DO = D // P
        self.FO = FF // P
        assert self.SB == 512 and self.MHs == P and self.HLs == 2


def build_program(cfg, self_cls, nsc, v_bias_zero=False):
    """Build + compile the SPMD program.

    self_cls: [NQ][NKs] entries SKIP / FULL / strip-index (identical on all
    cores).  nsc: number of cross strips (0 -> no cross mask work; else every
    cross block kb uses strip kb, strip data differs per core).
    """
    S, D, DK, SB, TQ = cfg.S, cfg.D, cfg.DK, cfg.SB, cfg.TQ
    NQ, NKs, NKc, DO, FO, G, H = (cfg.NQ, cfg.NKs, cfg.NKc, cfg.DO, cfg.FO,
                                  cfg.G, cfg.H)
    HLs, MHs, HLc, MHc, MOc = cfg.HLs, cfg.MHs, cfg.HLc, cfg.MHc, cfg.MOc
    nss = max(1, 1 + max((v for row in self_cls for v in row if v >= 0),
                         default=-1))
    groups4 = [list(range(g * G, (g + 1) * G)) for g in range(cfg.B)]
    groups8 = [list(range(cfg.n_cores))]

    nc = bacc.Bacc("TRN2", target_bir_lowering=False, debug=False,
                   num_devices=cfg.n_cores)

    def din(name, shape, dt):
        return nc.dram_tensor(name, shape, dt, kind="ExternalInput").ap()

    xT = din("xT", [P, DO, TQ], BF16)
    xck = din("xck", [P, DO, SB], F32)
    encT = din("encT", [P, DO, S], BF16)
    wq1 = din("wq1", [P, DO, MHs], BF16)
    wk1 = din("wk1", [P, DO, MHs], BF16)
    wv1 = din("wv1", [P, DO, MHs], BF16)
    bq1 = din("bq1", [P, 1], F32)
    bk1 = din("bk1", [P, 1], F32)
    bv1 = din("bv1", [1, MHs], BF16)
    wo1f = din("wo1f", [P, DO, D], BF16)
    bo1 = din("bo1", [P, DO], F32)
    wq2p = din("wq2p", [P, DO, DO, P], BF16)
    bq2 = din("bq2", [P, DO], F32)
    wk2 = din("wk2", [P, DO, MHc], BF16)
    bk2 = din("bk2", [P, MOc], F32)
    wv2 = din("wv2", [P, DO, MHc], BF16)
    bv2 = din("bv2", [1, MHc], BF16)
    wo2p = din("wo2p", [P, DO, DO, P], BF16)
    bo2 = din("bo2", [P, DO], F32)
    w1p = din("w1p", [P, FO, DO, P], BF16)
    b1 = din("b1", [P, FO], F32)
    w2p = din("w2p", [P, DO, FO, P], BF16)
    b2 = din("b2", [P, DO], F32)
    g1 = din("g1", [P, DO], F32)
    c1 = din("c1", [P, DO], F32)
    g2 = din("g2", [P, DO], F32)
    c2 = din("c2", [P, DO], F32)
    g3 = din("g3", [P, DO], F32)
    c3 = din("c3", [P, DO], F32)
    strS = din("strS", [P, nss, 512], BF16)
    strC = din("strC", [P, max(nsc, 1), SB], BF16)
    y = nc.dram_tensor("y", [DO, P, SB], F32, kind="ExternalOutput").ap()

    with tile.TileContext(nc) as tc:
        res_cm = tc.tile_pool(name="res", bufs=1)
        res = res_cm.__enter__()
        dram_cm = tc.tile_pool(name="dram", bufs=1, space="DRAM")
        dram = dram_cm.__enter__()
        ppmm_cm = tc.tile_pool(name="ppmm", bufs=3, space="PSUM")
        ppmm = ppmm_cm.__enter__()
        ppatt_cm = tc.tile_pool(name="ppatt", bufs=5, space="PSUM")
        ppatt = ppatt_cm.__enter__()

        _mm_prev = [None]

        def mm_chain(inst):
            # Total-order the final readers of rotating "mm" PSUM tiles so the
            # scheduler cannot invert drain order against slot capacity.
            if _mm_prev[0] is not None:
                add_dep_helper(inst.ins, _mm_prev[0].ins, sync=True,
                               reason="mm drain order")
            _mm_prev[0] = inst

        psm_cm = tc.tile_pool(name="psm", bufs=2)
        psm = psm_cm.__enter__()
        pC_cm = tc.tile_pool(name="pC", bufs=1)
        pC = pC_cm.__enter__()
        pw_cm = tc.tile_pool(name="pw", bufs=2)
        pw = pw_cm.__enter__()
        pln_cm = tc.tile_pool(name="pln", bufs=1)
        pln = pln_cm.__enter__()
        patt_cm = tc.tile_pool(name="patt", bufs=2)
        patt = patt_cm.__enter__()
        phB_cm = tc.tile_pool(name="phB", bufs=1)
        phB = phB_cm.__enter__()
        pio_x_cm = tc.tile_pool(name="pio_x", bufs=1)
        pio_x = pio_x_cm.__enter__()

        # ---- self-attn inputs first (x is on the critical path: QKV proj,
        # attention, AllToAll, O1, LN1, Q2 all chain after it) --------------
        XTt = pio_x.tile([P, DO, TQ], BF16, name="XTt")
        for o in range(DO):
            nc.sync.dma_start(XTt[:, o, :], xT[:, o, :])
        WQ1 = pio_x.tile([P, DO, MHs], BF16, name="WQ1")
        nc.sync.dma_start(WQ1[:], wq1)
        WK1 = pio_x.tile([P, DO, MHs], BF16, name="WK1")
        nc.sync.dma_start(WK1[:], wk1)
        WV1 = pio_x.tile([P, DO, MHs], BF16, name="WV1")
        nc.sync.dma_start(WV1[:], wv1)

        # ---- resident small tensors -------------------------------------
        def load_res(ap_in, shape, dt, name):
            t = res.tile(shape, dt, name=name)
            nc.sync.dma_start(t[:], ap_in)
            return t

        bq1t = load_res(bq1, [P, 1], F32, "bq1t")
        bk1t = load_res(bk1, [P, 1], F32, "bk1t")
        bv1t = load_res(bv1, [1, MHs], BF16, "bv1t")
        bo1t = load_res(bo1, [P, DO], F32, "bo1t")
        bq2t = load_res(bq2, [P, DO], F32, "bq2t")
        bk2t = load_res(bk2, [P, MOc], F32, "bk2t")
        bv2t = load_res(bv2, [1, MHc], BF16, "bv2t")
        bo2t = load_res(bo2, [P, DO], F32, "bo2t")
        b1t = load_res(b1, [P, FO], F32, "b1t")
        b2t = load_res(b2, [P, DO], F32, "b2t")
        g1t = load_res(g1, [P, DO], F32, "g1t")
        c1t = load_res(c1, [P, DO], F32, "c1t")
        g2t = load_res(g2, [P, DO], F32, "g2t")
        c2t = load_res(c2, [P, DO], F32, "c2t")
        g3t = load_res(g3, [P, DO], F32, "g3t")
        c3t = load_res(c3, [P, DO], F32, "c3t")
        strSt = load_res(strS, [P, nss, 512], BF16, "strSt")
        strCt = load_res(strC, [P, max(nsc, 1), SB], BF16, "strCt") \
            if nsc > 0 else None
        onesb = res.tile([1, P], BF16, name="onesb")
        nc.vector.memset(onesb[:], 1.0)
        ones128 = res.tile([P, P], BF16, name="ones128")
        nc.vector.memset(ones128[:], 1.0)
        shiftc = res.tile([P, 1], F32, name="shiftc")
        nc.vector.memset(shiftc[:], EXP_SHIFT)
        epsc = res.tile([P, 1], F32, name="epsc")
        nc.vector.memset(epsc[:], EPS)

        def qk_proj(out, wt, bias_t, src, mo_n, seq):
            for m in range(mo_n):
                for n in range(seq // 512):
                    ps = ppmm.tile([P, 512], F32, tag="mm", name="ps_qk")
                    for o in range(DO):
                        nc.tensor.matmul(
                            ps[:], wt[:, o, m * P:(m + 1) * P],
                            src[:, o, n * 512:(n + 1) * 512],
                            start=(o == 0), stop=(o == DO - 1))
                    mm_chain(nc.scalar.activation(
                        out[:, m, n * 512:(n + 1) * 512], ps[:],
                        AF.Identity, bias=bias_t[:, m:m + 1]))

        def v_proj(out, wt, bias_row, src, hl, seq):
            # out: [P(tok), seq//P, hl, 2*DK]; cols DK: stay for the ones blk
            mh = hl * DK
            for ms in range(seq // P):
                ps = ppmm.tile([P, 512], F32, tag="mm", name="ps_v")
                for o in range(DO):
                    nc.tensor.matmul(
                        ps[:, 0:mh], src[:, o, ms * P:(ms + 1) * P],
                        wt[:, o, :], start=(o == 0),
                        stop=(v_bias_zero and o == DO - 1))
                if not v_bias_zero:
                    nc.tensor.matmul(ps[:, 0:mh], onesb[0:1, :],
                                     bias_row[0:1, :], start=False, stop=True)
                mm_chain(nc.scalar.copy(
                    out[:, ms, :, 0:DK],
                    ps[:, 0:mh].rearrange("p (h d) -> p h d", h=hl)))

        # ---- self QKV projections ----------------------------------------
        # KT1z[z] is the K block for head z of the pair, zero-padded to the
        # full 128 partitions, so score matmuls run 128-contract like every
        # other matmul (no PE row-mode switches against the attV matmuls).
        QT1 = phB.tile([P, 1, TQ], BF16, name="QT1")
        KT1z = phB.tile([P, 2, TQ], BF16, name="KT1z")
        V1 = phB.tile([P, NKs, HLs, 2 * DK], BF16, name="V1")
        qk_proj(QT1, WQ1, bq1t, XTt, 1, TQ)
        nc.gpsimd.memset(KT1z[DK:2 * DK, 0, :], 0.0)
        nc.gpsimd.memset(KT1z[0:DK, 1, :], 0.0)
        for n in range(TQ // 512):
            ps = ppmm.tile([P, 512], F32, tag="mm", name="ps_k1")
            for o in range(DO):
                nc.tensor.matmul(
                    ps[:], WK1[:, o, :], XTt[:, o, n * 512:(n + 1) * 512],
                    start=(o == 0), stop=(o == DO - 1))
            mm_chain(nc.scalar.activation(
                KT1z[0:DK, 0, n * 512:(n + 1) * 512], ps[0:DK],
                AF.Identity, bias=bk1t[0:DK, 0:1]))
            mm_chain(nc.scalar.activation(
                KT1z[DK:2 * DK, 1, n * 512:(n + 1) * 512], ps[DK:2 * DK],
                AF.Identity, bias=bk1t[DK:2 * DK, 0:1]))
        v_proj(V1, WV1, bv1t, XTt, HLs, TQ)
        nc.gpsimd.memset(V1[:, :, :, DK:2 * DK], 1.0)
        pio_x_cm.__exit__(None, None, None)

        # ---- attention helper --------------------------------------------
        def attention_pair(qt_m, KT, QT, Vt, he, ho, qb, qw, cls_row,
                           strips, Xatt, xm, nk):
            """One (head-pair, q-block).  Even head lives on SBUF partitions
            0-63, odd head on 64-127.  Vt[..., DK:2DK] is an all-ones block,
            so each attV matmul emits 64 broadcast copies of the softmax
            denominator on partitions 64-127; normalization is pure DVE.
            Chunks are software-pipelined: scores/exp of chunk c+1 are issued
            before the attV matmuls of chunk c so the PE never waits on the
            scalar engine's exp."""
            allowed = [kb for kb in range(nk) if cls_row[kb] != SKIP]
            n = len(allowed)
            pa_e = ppatt.tile([P, 512], F32, tag="att", name="pa_e")
            pa_o = ppatt.tile([P, 512], F32, tag="att", name="pa_o")

            def do_scores(chunk, after=None):
                E = patt.tile([P, 2, ECH, 512], BF16, tag="E", name="E")
                mms = []
                for j, kb in enumerate(chunk):
                    for half in (0, 1):
                        ps = ppmm.tile([P, 512], F32, tag="mm", name="ps_s")
                        mm = nc.tensor.matmul(
                            ps[:, 0:qw],
                            KT[:, half, kb * P:(kb + 1) * P],
                            QT[:, qt_m, qb * qw:(qb + 1) * qw],
                            start=True, stop=True)
                        if after is not None and not mms:
                            add_dep_helper(mm.ins, after.ins, sync=True,
                                           reason="S after A block")
                        mms.append(mm)
                        if cls_row[kb] >= 0:
                            nc.vector.tensor_tensor(
                                ps[:, 0:qw], ps[:, 0:qw],
                                strips[:, cls_row[kb], 0:qw], ALU.add)
                        mm_chain(nc.scalar.activation(
                            E[:, half, j, 0:qw], ps[:, 0:qw],
                            AF.Exp, bias=shiftc[:, 0:1]))
                return E, mms[-1]

            def do_attv(E, chunk, c0, after=None):
                last = None
                for j, kb in enumerate(chunk):
                    mm = nc.tensor.matmul(pa_e[:, 0:qw], Vt[:, kb, he, :],
                                          E[:, 0, j, 0:qw],
                                          start=(c0 + j == 0),
                                          stop=(c0 + j == n - 1))
                    if after is not None and last is None:
                        add_dep_helper(mm.ins, after.ins, sync=True,
                                       reason="A after S block")
                    last = nc.tensor.matmul(pa_o[:, 0:qw], Vt[:, kb, ho, :],
                                            E[:, 1, j, 0:qw],
                                            start=(c0 + j == 0),
                                            stop=(c0 + j == n - 1))
                return last

            chunks = [allowed[c0:c0 + ECH] for c0 in range(0, n, ECH)]
            prevE, lastS = do_scores(chunks[0])
            prevA = None
            prev0 = 0
            for ci in range(1, len(chunks)):
                E2, lastS2 = do_scores(chunks[ci], after=prevA)
                prevA = do_attv(prevE, chunks[ci - 1], prev0, after=lastS2)
                prevE, lastS = E2, lastS2
                prev0 += len(chunks[ci - 1])
            do_attv(prevE, chunks[-1], prev0, after=None)

            # denominators -> bf16 SBUF (scalar), one fast-mode DVE recip
            den = psm.tile([P, 512], BF16, tag="den", name="den")
            nc.vector.tensor_copy(den[0:DK, 0:qw], pa_e[DK:2 * DK, 0:qw])
            nc.vector.tensor_copy(den[DK:2 * DK, 0:qw], pa_o[DK:2 * DK, 0:qw])
            rec = psm.tile([P, 512], BF16, tag="rec", name="rec")
            with nc.allow_low_precision(reason="softmax denom recip in bf16"):
                nc.vector.reciprocal(rec[:, 0:qw], den[:, 0:qw])
            nc.vector.tensor_tensor(
                Xatt[0:DK, xm, 0:qw],
                pa_e[0:DK, 0:qw], rec[0:DK, 0:qw], ALU.mult)
            nc.vector.tensor_tensor(
                Xatt[DK:2 * DK, xm, 0:qw],
                pa_o[0:DK, 0:qw], rec[DK:2 * DK, 0:qw], ALU.mult)

        # ---- self attention + AllToAll context exchange ------------------
        # cross K/V projection + its AllGather are emitted after qb=1 so the
        # enc_out load hides under early attention and the AG under the rest.
        HB = SB // 2
        a2a_inA = dram.tile([cfg.n_cores, P, HB], BF16, name="a2a_inA")
        a2a_inB = dram.tile([cfg.n_cores, P, HB], BF16, name="a2a_inB")

        def self_qb(qb):
            xst = pln.tile([P, 1, SB], BF16, tag="xst", name="xst", bufs=2)
            attention_pair(0, KT1z, QT1, V1, 0, 1, qb, 512, self_cls[qb],
                           strSt, xst, 0, NKs)
            nc.sync.dma_start(a2a_inA[qb], xst[:, 0, 0:HB])
            nc.sync.dma_start(a2a_inB[qb], xst[:, 0, HB:SB])

        qb_order = sorted(range(NQ), key=lambda q: -sum(
            1 for v in self_cls[q] if v != SKIP))
        self_qb(qb_order[0])
        self_qb(qb_order[1])
        # ---- cross K/V projections + AllGather (hides under self-attn) ---
        pA_kv_cm = tc.tile_pool(name="pA_kv", bufs=1)
        pA_kv = pA_kv_cm.__enter__()
        pio_e_cm = tc.tile_pool(name="pio_e", bufs=1)
        pio_e = pio_e_cm.__enter__()
        ENCTt = pio_e.tile([P, DO, S], BF16, name="ENCTt")
        for o in range(DO):
            nc.sync.dma_start(ENCTt[:, o, :], encT[:, o, :])
        WK2 = pio_e.tile([P, DO, MHc], BF16, name="WK2")
        nc.sync.dma_start(WK2[:], wk2)
        WV2 = pio_e.tile([P, DO, MHc], BF16, name="WV2")
        nc.sync.dma_start(WV2[:], wv2)
        KT2l = pA_kv.tile([P, MOc, S], BF16, name="KT2l")
        V2l = pA_kv.tile([P, NKc, HLc, DK], BF16, name="V2l")
        qk_proj(KT2l, WK2, bk2t, ENCTt, MOc, S)
        for ms in range(S // P):
            ps = ppmm.tile([P, 512], F32, tag="mm", name="ps_v2")
            for o in range(DO):
                nc.tensor.matmul(
                    ps[:, 0:MHc], ENCTt[:, o, ms * P:(ms + 1) * P],
                    WV2[:, o, :], start=(o == 0),
                    stop=(v_bias_zero and o == DO - 1))
            if not v_bias_zero:
                nc.tensor.matmul(ps[:, 0:MHc], onesb[0:1, :],
                                 bv2t[0:1, :], start=False, stop=True)
            mm_chain(nc.scalar.copy(
                V2l[:, ms, :, :],
                ps[:, 0:MHc].rearrange("p (h d) -> p h d", h=HLc)))
        pio_e_cm.__exit__(None, None, None)

        KSZ = MOc * S
        VSZ = NKc * HLc * DK
        kv_loc = dram.tile([P, KSZ + VSZ], BF16, name="kv_loc")
        nc.sync.dma_start(kv_loc[:, 0:KSZ],
                          KT2l[:].rearrange("p m s -> p (m s)"))
        nc.sync.dma_start(kv_loc[:, KSZ:KSZ + VSZ],
                          V2l[:].rearrange("p k h d -> p (k h d)"))
        kv_ag = dram.tile([G, P, KSZ + VSZ], BF16, name="kv_ag")
        nc.gpsimd.collective_compute(
            "AllGather", ALU.bypass, replica_groups=groups4,
            ins=[kv_loc.opt()], outs=[kv_ag.opt()])
        pA_kv_cm.__exit__(None, None, None)

        # post-attention working set (region reuse; DMAs run under attn) ---
        pO1_cm = tc.tile_pool(name="pO1", bufs=1)
        pO1 = pO1_cm.__enter__()
        xckt = pO1.tile([P, DO, SB], F32, name="xckt")
        nc.sync.dma_start(xckt[:], xck)
        WO1t = pO1.tile([P, DO, D], BF16, name="WO1t")
        nc.sync.dma_start(WO1t[:], wo1f)

        for qb in qb_order[2:]:
            self_qb(qb)
        a2a_outA = dram.tile([cfg.n_cores, P, HB], BF16, name="a2a_outA")
        a2a_outB = dram.tile([cfg.n_cores, P, HB], BF16, name="a2a_outB")
        nc.gpsimd.collective_compute(
            "AllToAll", ALU.bypass, replica_groups=groups8,
            ins=[a2a_inA.opt()], outs=[a2a_outA.opt()])
        nc.gpsimd.collective_compute(
            "AllToAll", ALU.bypass, replica_groups=groups8,
            ins=[a2a_inB.opt()], outs=[a2a_outB.opt()])
        XA = pO1.tile([P, DO, SB], BF16, name="XA")
        for j in range(cfg.n_cores):
            nc.sync.dma_start(XA[:, j, 0:HB], a2a_outA[j])
        for j in range(cfg.n_cores):
            nc.sync.dma_start(XA[:, j, HB:SB], a2a_outB[j])

        # ---- LN helpers (stats pre-broadcast via all-ones stationary;
        # per-m stat matmuls are emitted lag-one inside the producer loops
        # so the PE never drains between a projection and its LN) ----------
        def ln_stats_begin():
            psA = ppatt.tile([P, 512], F32, tag="att", name="psA")
            psB = ppatt.tile([P, 512], F32, tag="att", name="psB")
            return psA, psB

        def ln_stat_m(st, S_sb, m, W):
            psA, psB = st
            sbf = pln.tile([P, W], BF16, tag="sbf", name="sbf", bufs=2)
            sq = pln.tile([P, W], BF16, tag="sq", name="sq", bufs=2)
            nc.vector.tensor_copy(sbf[:], S_sb[:, m])
            nc.vector.tensor_mul(sq[:], S_sb[:, m], S_sb[:, m])
            nc.tensor.matmul(psA[:, 0:W], ones128[:], sbf[:],
                             start=(m == 0), stop=(m == DO - 1))
            nc.tensor.matmul(psB[:, 0:W], ones128[:], sq[:],
                             start=(m == 0), stop=(m == DO - 1))

        def ln_finish(st, S_sb, gt, ct, out_f32, out_bf16, W):
            psA, psB = st
            mu = psm.tile([P, 512], F32, tag="stat", name="mu", bufs=4)
            nc.scalar.activation(mu[:, 0:W], psA[:, 0:W], AF.Copy,
                                 scale=1.0 / D)
            e2 = psm.tile([P, 512], F32, tag="stat", name="e2", bufs=4)
            nc.scalar.activation(e2[:, 0:W], psB[:, 0:W], AF.Copy,
                                 scale=1.0 / D)
            musq = psm.tile([P, 512], F32, tag="stat", name="musq", bufs=4)
            nc.vector.tensor_mul(musq[:, 0:W], mu[:, 0:W], mu[:, 0:W])
            nc.vector.tensor_sub(musq[:, 0:W], musq[:, 0:W], e2[:, 0:W])
            # musq now holds -var; sd = sqrt(-musq + eps); rstd = 1/sd
            sd = psm.tile([P, 512], BF16, tag="sdb", name="sd", bufs=2)
            nc.scalar.activation(sd[:, 0:W], musq[:, 0:W], AF.Sqrt,
                                 bias=epsc[:, 0:1], scale=-1.0)
            rstd = psm.tile([P, 512], BF16, tag="sdb", name="rstd", bufs=2)
            with nc.allow_low_precision(reason="LN rstd in bf16"):
                nc.vector.reciprocal(rstd[:, 0:W], sd[:, 0:W])
            mr = psm.tile([P, 512], F32, tag="stat", name="mr", bufs=4)
            nc.vector.tensor_mul(mr[:, 0:W], mu[:, 0:W], rstd[:, 0:W])
            for m in range(DO):
                t2 = pln.tile([P, W], F32, tag="t2", name="t2", bufs=2)
                nc.vector.tensor_mul(t2[:], S_sb[:, m], rstd[:, 0:W])
                nc.vector.tensor_sub(t2[:], t2[:], mr[:, 0:W])
                nc.scalar.activation(out_f32[:, m], t2[:], AF.Identity,
                                     bias=ct[:, m:m + 1], scale=gt[:, m:m + 1])
                if out_bf16 is not None:
                    nc.vector.tensor_copy(out_bf16[:, m], out_f32[:, m])

        # ---- O1 projection (full D) + residual + LN1 ---------------------
        S1 = pO1.tile([P, DO, SB], F32, name="S1")
        st1 = ln_stats_begin()
        for h in range(2):
            cr = slice(h * HB, (h + 1) * HB)
            for m in range(DO):
                ps = ppmm.tile([P, 512], F32, tag="mm", name="ps_o1")
                for j in range(DO):
                    nc.tensor.matmul(
                        ps[:, 0:HB], WO1t[:, j, m * P:(m + 1) * P],
                        XA[:, j, cr], start=(j == 0), stop=(j == DO - 1))
                mm_chain(nc.vector.scalar_tensor_tensor(
                    S1[:, m, cr], ps[:, 0:HB], bo1t[:, m:m + 1],
                    xckt[:, m, cr], ALU.add, ALU.add))
                if h == 1 and m > 0:
                    ln_stat_m(st1, S1, m - 1, SB)
        ln_stat_m(st1, S1, DO - 1, SB)
        X2f = pC.tile([P, DO, SB], F32, name="X2f")
        X2b = pC.tile([P, DO, SB], BF16, name="X2b")
        ln_finish(st1, S1, g1t, c1t, X2f, X2b, SB)
        pO1_cm.__exit__(None, None, None)
        phB_cm.__exit__(None, None, None)

        # cross-attn K/V staging enters now: its DMAs overlap LN1/Q2proj
        pva_cm = tc.tile_pool(name="pva", bufs=1)
        pva = pva_cm.__enter__()
        pkt_cm = tc.tile_pool(name="pkt", bufs=1)
        pkt = pkt_cm.__enter__()

        # ---- cross-attn Q projection (token-sharded, all heads) ----------
        Q2T = pC.tile([P, DO, SB], BF16, name="Q2T")
        for m in range(DO):
            wq2t = pw.tile([P, DO, P], BF16, tag="wsm", name="wq2t")
            nc.sync.dma_start(wq2t[:], wq2p[:, m])
            ps = ppmm.tile([P, 512], F32, tag="mm", name="ps_q2")
            for o in range(DO):
                nc.tensor.matmul(ps[:, 0:SB], wq2t[:, o, :], X2b[:, o, :],
                                 start=(o == 0), stop=(o == DO - 1))
            mm_chain(nc.scalar.activation(Q2T[:, m, :], ps[:, 0:SB],
                                          AF.Identity, bias=bq2t[:, m:m + 1]))

        # ---- cross attention (two head-half passes, double-buffered) -----
        Xatt2 = pC.tile([P, DO, SB], BF16, name="Xatt2")
        cross_cls = [kb if nsc > 0 else FULL for kb in range(NKc)]
        HH = H // 2
        ktpz = [pkt.tile([P, 2, S], BF16, name=f"ktpz{i}") for i in range(2)]
        for i in range(2):
            nc.gpsimd.memset(ktpz[i][DK:2 * DK, 0, :], 0.0)
            nc.gpsimd.memset(ktpz[i][0:DK, 1, :], 0.0)
        for half in range(2):
            Va = pva.tile([P, NKc, HH, 2 * DK], BF16, tag="va", name="Va",
                          bufs=2)
            h0 = half * HH
            for r in range(G):
                lo = max(h0, r * HLc)
                hi = min(h0 + HH, (r + 1) * HLc)
                if lo >= hi:
                    continue
                for h in range(lo, hi):
                    nc.sync.dma_start(
                        Va[:, :, h - h0, 0:DK],
                        kv_ag[r, :, KSZ:KSZ + VSZ].rearrange(
                            "p (k h d) -> p k h d", k=NKc, h=HLc)
                        [:, :, h - r * HLc, :])
            nc.gpsimd.memset(Va[:, :, :, DK:2 * DK], 1.0)
            for hp in range(h0 // 2, (h0 + HH) // 2):
                r, mo = divmod(hp, MOc)
                kz = ktpz[hp % 2]
                nc.sync.dma_start(kz[0:DK, 0, :],
                                  kv_ag[r, 0:DK, mo * S:(mo + 1) * S])
                nc.sync.dma_start(kz[DK:2 * DK, 1, :],
                                  kv_ag[r, DK:2 * DK, mo * S:(mo + 1) * S])
                attention_pair(hp, kz, Q2T, Va, 2 * hp - h0,
                               2 * hp + 1 - h0, 0, SB, cross_cls, strCt,
                               Xatt2, hp, NKc)
        pkt_cm.__exit__(None, None, None)
        pva_cm.__exit__(None, None, None)
        pff_cm = tc.tile_pool(name="pff", bufs=1)
        pff = pff_cm.__enter__()

        # ---- cross O-projection + residual (in-place over X2f) + LN2 ----
        st2 = ln_stats_begin()
        for m in range(DO):
            wo2t = pw.tile([P, DO, P], BF16, tag="wsm", name="wo2t")
            nc.sync.dma_start(wo2t[:], wo2p[:, m])
            ps = ppmm.tile([P, 512], F32, tag="mm", name="ps_o2")
            for o in range(DO):
                nc.tensor.matmul(ps[:, 0:SB], wo2t[:, o, :], Xatt2[:, o, :],
                                 start=(o == 0), stop=(o == DO - 1))
            mm_chain(nc.vector.scalar_tensor_tensor(
                X2f[:, m], ps[:, 0:SB], bo2t[:, m:m + 1], X2f[:, m],
                ALU.add, ALU.add))
            if m > 0:
                ln_stat_m(st2, X2f, m - 1, SB)
        ln_stat_m(st2, X2f, DO - 1, SB)
        X4f = pff.tile([P, DO, SB], F32, name="X4f")
        X4b = pff.tile([P, DO, SB], BF16, name="X4b")
        ln_finish(st2, X2f, g2t, c2t, X4f, X4b, SB)

        # ---- FFN (two FO-halves; accumulate into X4f in place) ----------
        st3 = ln_stats_begin()
        FH = FO // 2
        for half in range(2):
            f0 = half * FH
            Ht = pff.tile([P, FH, SB], BF16, tag="Ht", name="Ht")
            for mf in range(FH):
                w1t = pw.tile([P, DO, P], BF16, tag="wsm", name="w1t")
                nc.sync.dma_start(w1t[:], w1p[:, f0 + mf])
                ps = ppmm.tile([P, 512], F32, tag="mm", name="ps_f1")
                for o in range(DO):
                    nc.tensor.matmul(ps[:, 0:SB], w1t[:, o, :], X4b[:, o, :],
                                     start=(o == 0), stop=(o == DO - 1))
                mm_chain(nc.scalar.activation(
                    Ht[:, mf, :], ps[:, 0:SB], AF.Relu,
                    bias=b1t[:, f0 + mf:f0 + mf + 1]))
            for m in range(DO):
                w2t = pw.tile([P, FH, P], BF16, tag="w2", name="w2t")
                nc.sync.dma_start(w2t[:], w2p[:, m, f0:f0 + FH, :])
                ps = ppmm.tile([P, 512], F32, tag="mm", name="ps_f2")
                for of in range(FH):
                    nc.tensor.matmul(ps[:, 0:SB], w2t[:, of, :], Ht[:, of, :],
                                     start=(of == 0), stop=(of == FH - 1))
                if half == 0:
                    mm_chain(nc.vector.scalar_tensor_tensor(
                        X4f[:, m], ps[:, 0:SB], b2t[:, m:m + 1], X4f[:, m],
                        ALU.add, ALU.add))
                else:
                    mm_chain(nc.vector.tensor_add(
                        X4f[:, m], X4f[:, m], ps[:, 0:SB]))
                    if m > 0:
                        ln_stat_m(st3, X4f, m - 1, SB)
        ln_stat_m(st3, X4f, DO - 1, SB)
        ln_finish(st3, X4f, g3t, c3t, X4f, None, SB)
        for m in range(DO):
            nc.sync.dma_start(y[m], X4f[:, m])

        pff_cm.__exit__(None, None, None)
        patt_cm.__exit__(None, None, None)
        pln_cm.__exit__(None, None, None)
        pw_cm.__exit__(None, None, None)
        pC_cm.__exit__(None, None, None)
        psm_cm.__exit__(None, None, None)
        ppatt_cm.__exit__(None, None, None)
        ppmm_cm.__exit__(None, None, None)
        dram_cm.__exit__(None, None, None)
        res_cm.__exit__(None, None, None)

    nc.compile()
    return nc


# ---------------------------------------------------------------------------
# host side
# ---------------------------------------------------------------------------

def _pack_ko(a):
    """[K, F] -> [128, K//128, F] (contract dim on partitions)."""
    K, F = a.shape
    return np.ascontiguousarray(a.reshape(K // P, P, F).transpose(1, 0, 2))


def _pack_vec(v, n):
    return np.ascontiguousarray(np.asarray(v, np.float32).reshape(n, P).T)


def classify_self(mask, NQ, NK):
    """mask [S, S] bool (q, k). Returns cls [NQ][NK] and strips [128, nss, 512]."""
    cls = [[FULL] * NK for _ in range(NQ)]
    strips = []
    keys = {}
    for qb in range(NQ):
        for kb in range(NK):
            blk = mask[qb * 512:(qb + 1) * 512, kb * P:(kb + 1) * P]
            if blk.all():
                cls[qb][kb] = FULL
            elif not blk.any():
                cls[qb][kb] = SKIP
            else:
                key = blk.tobytes()
                if key not in keys:
                    keys[key] = len(strips)
                    strips.append(np.where(blk.T, np.float32(0),
                                           np.float32(NEG)))
                cls[qb][kb] = keys[key]
    if strips:
        arr = np.stack(strips, 0).transpose(1, 0, 2)
    else:
        arr = np.zeros((P, 1, 512), np.float32)
    return cls, np.ascontiguousarray(arr).astype(bfloat16)


_CACHE = {}


def kernel(**inputs):
    cfg = Cfg(S=2048, D=1024, H=16, FF=4096, TP=4, B=2)
    return _run(cfg, inputs)


def _run(cfg, inputs, trace=False):
    S, D, G, B, SB, DO = cfg.S, cfg.D, cfg.G, cfg.B, cfg.SB, cfg.DO
    MHs, MHc, MOc, NQ, NKs, NKc = (cfg.MHs, cfg.MHc, cfg.MOc, cfg.NQ,
                                   cfg.NKs, cfg.NKc)
    f32 = np.float32
    bf = bfloat16
    tgt = np.asarray(inputs["tgt_mask"])[0, 0] != 0
    src = np.asarray(inputs["src_mask"])[0, 0] != 0

    # per-batch causal classification, composed block-diagonally over B
    clsb, strS = classify_self(tgt, S // 512, S // P)
    nqb, nkb = S // 512, S // P
    self_cls = [[SKIP] * NKs for _ in range(NQ)]
    for qb in range(NQ):
        for kb in range(NKs):
            if qb // nqb == kb // nkb:
                self_cls[qb][kb] = clsb[qb % nqb][kb % nkb]
    nsc = 0 if src.all() else NKc

    v_bias_zero = (not np.asarray(inputs["m1_bv"]).any()) and \
        (not np.asarray(inputs["m2_bv"]).any())
    key = (cfg.S, cfg.D, cfg.H, cfg.FF, cfg.G, cfg.B,
           tuple(map(tuple, self_cls)), nsc, v_bias_zero)
    if key not in _CACHE:
        _CACHE[key] = build_program(cfg, self_cls, nsc, v_bias_zero)
    nc = _CACHE[key]

    x = np.asarray(inputs["x"], f32)
    enc = np.asarray(inputs["enc_out"], f32)
    w1 = np.asarray(inputs["ff_w1"], f32)
    w2 = np.asarray(inputs["ff_w2"], f32)
    wq2 = np.asarray(inputs["m2_wq"], f32)
    wo2 = np.asarray(inputs["m2_wo"], f32)

    # xT: both batches concatenated on the token axis (batch-major)
    xT_full = np.concatenate([x[0], x[1]], axis=0).T  # [D, TQ]
    xT_pack = np.ascontiguousarray(
        xT_full.reshape(DO, P, cfg.TQ).transpose(1, 0, 2)).astype(bf)

    shared = {
        "xT": xT_pack,
        "wo1f": _pack_ko(np.asarray(inputs["m1_wo"], f32)).astype(bf),
        "wq2p": np.ascontiguousarray(
            wq2.reshape(DO, P, DO, P).transpose(1, 2, 0, 3)).astype(bf),
        "bq2": _pack_vec(inputs["m2_bq"], DO),
        "wo2p": np.ascontiguousarray(
            wo2.reshape(DO, P, DO, P).transpose(1, 2, 0, 3)).astype(bf),
        "bo2": _pack_vec(inputs["m2_bo"], DO),
        "bo1": _pack_vec(inputs["m1_bo"], DO),
        "w1p": np.ascontiguousarray(
            w1.reshape(DO, P, cfg.FO, P).transpose(1, 2, 0, 3)).astype(bf),
        "b1": _pack_vec(inputs["ff_b1"], cfg.FO),
        "w2p": np.ascontiguousarray(
            w2.reshape(cfg.FO, P, DO, P).transpose(1, 2, 0, 3)).astype(bf),
        "b2": _pack_vec(inputs["ff_b2"], DO),
        "g1": _pack_vec(inputs["ln1_g"], DO),
        "c1": _pack_vec(inputs["ln1_b"], DO),
        "g2": _pack_vec(inputs["ln2_g"], DO),
        "c2": _pack_vec(inputs["ln2_b"], DO),
        "g3": _pack_vec(inputs["ln3_g"], DO),
        "c3": _pack_vec(inputs["ln3_b"], DO),
        "strS": strS,
    }

    in_maps = []
    for c in range(cfg.n_cores):
        b, r = divmod(c, G)
        xTb = np.ascontiguousarray(x[b].T)
        encTb = np.ascontiguousarray(enc[b].T)
        m = dict(shared)
        m["xck"] = np.ascontiguousarray(
            xTb[:, r * SB:(r + 1) * SB].reshape(DO, P, SB).transpose(1, 0, 2))
        m["encT"] = np.ascontiguousarray(
            encTb.reshape(DO, P, S).transpose(1, 0, 2)).astype(bf)
        # self-attn: 2 heads per core (TP=8 over heads)
        hs = slice(c * MHs, (c + 1) * MHs)
        m["wq1"] = _pack_ko(np.asarray(inputs["m1_wq"], f32)[:, hs]).astype(bf)
        m["wk1"] = _pack_ko(np.asarray(inputs["m1_wk"], f32)[:, hs]).astype(bf)
        m["wv1"] = _pack_ko(np.asarray(inputs["m1_wv"], f32)[:, hs]).astype(bf)
        m["bq1"] = _pack_vec(np.asarray(inputs["m1_bq"], f32)[hs], 1)
        m["bk1"] = _pack_vec(np.asarray(inputs["m1_bk"], f32)[hs], 1)
        m["bv1"] = np.asarray(inputs["m1_bv"], f32)[hs].reshape(1, MHs).astype(bf)
        # cross-attn K/V: 4 heads per group rank
        hc = slice(r * MHc, (r + 1) * MHc)
        m["wk2"] = _pack_ko(np.asarray(inputs["m2_wk"], f32)[:, hc]).astype(bf)
        m["wv2"] = _pack_ko(np.asarray(inputs["m2_wv"], f32)[:, hc]).astype(bf)
        m["bk2"] = _pack_vec(np.asarray(inputs["m2_bk"], f32)[hc], MOc)
        m["bv2"] = np.asarray(inputs["m2_bv"], f32)[hc].reshape(1, MHc).astype(bf)
        if nsc > 0:
            blks = []
            for kb in range(NKc):
                blk = src[r * SB:(r + 1) * SB, kb * P:(kb + 1) * P]
                blks.append(np.where(blk.T, f32(0), f32(NEG)))
            m["strC"] = np.ascontiguousarray(
                np.stack(blks, 0).transpose(1, 0, 2)).astype(bf)
        else:
            m["strC"] = np.zeros((P, 1, SB), bf)
        in_maps.append(m)

    res = bass_utils.run_bass_kernel_spmd(
        nc, in_maps, core_ids=list(range(cfg.n_cores)), trace=trace)

    out = np.empty((B, S, D), f32)
    for c in range(cfg.n_cores):
        b, r = divmod(c, G)
        yv = res.results[c]["y"]
        out[b, r * SB:(r + 1) * SB, :] = yv.transpose(2, 0, 1).reshape(SB, D)
    if trace:
        return out, res
    return out


# revision 18
# speedup vs baseline: 1.0542x; 1.0542x over previous
"""Trainium2 Bass kernel for a transformer decoder layer (self-attn + cross-attn + FFN).

Distribution over 8 NeuronCores:
  * self-attention: TP=8 over heads (2 heads/core) with BOTH batches
    concatenated on the token axis (4096 token-instances per core); the
    attention context is exchanged with a single 8-rank AllToAll (1MB) so each
    core ends up with all 1024 context features for its 512 tokens, then the
    full O-projection runs locally (no ReduceScatter).
  * cross-attention K/V: computed head-sharded per 4-core batch group from
    enc_out FIRST, and AllGathered under self-attention (off critical path).
    Cross-attention itself runs token-sharded with all 16 heads per core.
  * LayerNorms + FFN: token-sharded (512 tokens/core), no further comm.

All activations are feature-major ([D, seq]) on chip; matmuls are bf16 with
fp32 PSUM accumulation; softmax uses a constant exp-shift (exact for softmax)
with scores^T ([k, q]) layout.  The stationary [V | ones*64] trick makes each
attV matmul produce 64 broadcast copies of the softmax denominator, so
normalization is pure DVE (no PE broadcast matmuls).  LN statistics use an
all-ones [P,128] stationary so mean/var arrive pre-broadcast on 128
partitions.
"""

import sys

sys.path.insert(0, "/opt/trn_rl_repo")

import numpy as np
from ml_dtypes import bfloat16

import concourse.bass as bass
import concourse.mybir as mybir
import concourse.tile as tile
from concourse.tile_rust import add_dep_helper
from concourse import bacc
from concourse import bass_utils

F32 = mybir.dt.float32
BF16 = mybir.dt.bfloat16
ALU = mybir.AluOpType
AF = mybir.ActivationFunctionType

P = 128
EPS = 1e-5
EXP_SHIFT = -12.0  # exp(s + EXP_SHIFT): overflow headroom, exact for softmax
NEG = -1e9

SKIP = -2
FULL = -1

ECH = 4  # k-blocks per attention-weight tile


class Cfg:
    def __init__(self, S, D, H, FF, TP, B):
        self.S, self.D, self.H, self.FF, self.B = S, D, H, FF, B
        self.G = TP                  # cross-attn group size (per batch)
        self.DK = 64
        self.n_cores = TP * B        # 8
        self.TQ = B * S              # self-attn token instances
        self.HLs = H // self.n_cores # self local heads (2)
        self.MHs = self.HLs * self.DK
        self.NQ = self.TQ // 512     # self q blocks (8)
        self.NKs = self.TQ // P      # self k blocks (32)
        self.HLc = H // TP           # cross local heads for the AG (4)
        self.MHc = self.HLc * self.DK
        self.MOc = self.MHc // P
        self.NKc = S // P            # cross k blocks (16)
        self.SB = S // TP            # token chunk (512)
        self.DO = D // P
        self.FO = FF // P
        assert self.SB == 512 and self.MHs == P and self.HLs == 2


def build_program(cfg, self_cls, nsc, v_bias_zero=False):
    """Build + compile the SPMD program.

    self_cls: [NQ][NKs] entries SKIP / FULL / strip-index (identical on all
    cores).  nsc: number of cross strips (0 -> no cross mask work; else every
    cross block kb uses strip kb, strip data differs per core).
    """
    S, D, DK, SB, TQ = cfg.S, cfg.D, cfg.DK, cfg.SB, cfg.TQ
    NQ, NKs, NKc, DO, FO, G, H = (cfg.NQ, cfg.NKs, cfg.NKc, cfg.DO, cfg.FO,
                                  cfg.G, cfg.H)
    HLs, MHs, HLc, MHc, MOc = cfg.HLs, cfg.MHs, cfg.HLc, cfg.MHc, cfg.MOc
    nss = max(1, 1 + max((v for row in self_cls for v in row if v >= 0),
                         default=-1))
    groups4 = [list(range(g * G, (g + 1) * G)) for g in range(cfg.B)]
    groups8 = [list(range(cfg.n_cores))]

    nc = bacc.Bacc("TRN2", target_bir_lowering=False, debug=False,
                   num_devices=cfg.n_cores)

    def din(name, shape, dt):
        return nc.dram_tensor(name, shape, dt, kind="ExternalInput").ap()

    xT = din("xT", [P, DO, TQ], BF16)
    xck = din("xck", [P, DO, SB], F32)
    encT = din("encT", [P, DO, S], BF16)
    wq1 = din("wq1", [P, DO, MHs], BF16)
    wk1 = din("wk1", [P, DO, MHs], BF16)
    wv1 = din("wv1", [P, DO, MHs], BF16)
    bq1 = din("bq1", [P, 1], F32)
    bk1 = din("bk1", [P, 1], F32)
    bv1 = din("bv1", [1, MHs], BF16)
    wo1f = din("wo1f", [P, DO, D], BF16)
    bo1 = din("bo1", [P, DO], F32)
    wq2p = din("wq2p", [P, DO, DO, P], BF16)
    bq2 = din("bq2", [P, DO], F32)
    wk2 = din("wk2", [P, DO, MHc], BF16)
    bk2 = din("bk2", [P, MOc], F32)
    wv2 = din("wv2", [P, DO, MHc], BF16)
    bv2 = din("bv2", [1, MHc], BF16)
    wo2p = din("wo2p", [P, DO, DO, P], BF16)
    bo2 = din("bo2", [P, DO], F32)
    w1p = din("w1p", [P, FO, DO, P], BF16)
    b1 = din("b1", [P, FO], F32)
    w2p = din("w2p", [P, DO, FO, P], BF16)
    b2 = din("b2", [P, DO], F32)
    g1 = din("g1", [P, DO], F32)
    c1 = din("c1", [P, DO], F32)
    g2 = din("g2", [P, DO], F32)
    c2 = din("c2", [P, DO], F32)
    g3 = din("g3", [P, DO], F32)
    c3 = din("c3", [P, DO], F32)
    strS = din("strS", [P, nss, 512], BF16)
    strC = din("strC", [P, max(nsc, 1), SB], BF16)
    y = nc.dram_tensor("y", [DO, P, SB], F32, kind="ExternalOutput").ap()

    with tile.TileContext(nc) as tc:
        res_cm = tc.tile_pool(name="res", bufs=1)
        res = res_cm.__enter__()
        dram_cm = tc.tile_pool(name="dram", bufs=1, space="DRAM")
        dram = dram_cm.__enter__()
        ppmm_cm = tc.tile_pool(name="ppmm", bufs=3, space="PSUM")
        ppmm = ppmm_cm.__enter__()
        ppatt_cm = tc.tile_pool(name="ppatt", bufs=5, space="PSUM")
        ppatt = ppatt_cm.__enter__()

        _mm_prev = [None]

        def mm_chain(inst):
            # Total-order the final readers of rotating "mm" PSUM tiles so the
            # scheduler cannot invert drain order against slot capacity.
            if _mm_prev[0] is not None:
                add_dep_helper(inst.ins, _mm_prev[0].ins, sync=True,
                               reason="mm drain order")
            _mm_prev[0] = inst

        psm_cm = tc.tile_pool(name="psm", bufs=2)
        psm = psm_cm.__enter__()
        pC_cm = tc.tile_pool(name="pC", bufs=1)
        pC = pC_cm.__enter__()
        pw_cm = tc.tile_pool(name="pw", bufs=2)
        pw = pw_cm.__enter__()
        pln_cm = tc.tile_pool(name="pln", bufs=1)
        pln = pln_cm.__enter__()
        patt_cm = tc.tile_pool(name="patt", bufs=2)
        patt = patt_cm.__enter__()
        phB_cm = tc.tile_pool(name="phB", bufs=1)
        phB = phB_cm.__enter__()
        pio_x_cm = tc.tile_pool(name="pio_x", bufs=1)
        pio_x = pio_x_cm.__enter__()

        # ---- self-attn inputs first (x is on the critical path: QKV proj,
        # attention, AllToAll, O1, LN1, Q2 all chain after it) --------------
        XTt = pio_x.tile([P, DO, TQ], BF16, name="XTt")
        for o in range(DO):
            nc.sync.dma_start(XTt[:, o, :], xT[:, o, :])
        WQ1 = pio_x.tile([P, DO, MHs], BF16, name="WQ1")
        nc.sync.dma_start(WQ1[:], wq1)
        WK1 = pio_x.tile([P, DO, MHs], BF16, name="WK1")
        nc.sync.dma_start(WK1[:], wk1)
        WV1 = pio_x.tile([P, DO, MHs], BF16, name="WV1")
        nc.sync.dma_start(WV1[:], wv1)

        # ---- resident small tensors -------------------------------------
        def load_res(ap_in, shape, dt, name):
            t = res.tile(shape, dt, name=name)
            nc.sync.dma_start(t[:], ap_in)
            return t

        bq1t = load_res(bq1, [P, 1], F32, "bq1t")
        bk1t = load_res(bk1, [P, 1], F32, "bk1t")
        bv1t = load_res(bv1, [1, MHs], BF16, "bv1t")
        bo1t = load_res(bo1, [P, DO], F32, "bo1t")
        bq2t = load_res(bq2, [P, DO], F32, "bq2t")
        bk2t = load_res(bk2, [P, MOc], F32, "bk2t")
        bv2t = load_res(bv2, [1, MHc], BF16, "bv2t")
        bo2t = load_res(bo2, [P, DO], F32, "bo2t")
        b1t = load_res(b1, [P, FO], F32, "b1t")
        b2t = load_res(b2, [P, DO], F32, "b2t")
        g1t = load_res(g1, [P, DO], F32, "g1t")
        c1t = load_res(c1, [P, DO], F32, "c1t")
        g2t = load_res(g2, [P, DO], F32, "g2t")
        c2t = load_res(c2, [P, DO], F32, "c2t")
        g3t = load_res(g3, [P, DO], F32, "g3t")
        c3t = load_res(c3, [P, DO], F32, "c3t")
        strSt = load_res(strS, [P, nss, 512], BF16, "strSt")
        strCt = load_res(strC, [P, max(nsc, 1), SB], BF16, "strCt") \
            if nsc > 0 else None
        onesb = res.tile([1, P], BF16, name="onesb")
        nc.vector.memset(onesb[:], 1.0)
        ones128 = res.tile([P, P], BF16, name="ones128")
        nc.vector.memset(ones128[:], 1.0)
        shiftc = res.tile([P, 1], F32, name="shiftc")
        nc.vector.memset(shiftc[:], EXP_SHIFT)
        epsc = res.tile([P, 1], F32, name="epsc")
        nc.vector.memset(epsc[:], EPS)

        def qk_proj(out, wt, bias_t, src, mo_n, seq):
            for m in range(mo_n):
                for n in range(seq // 512):
                    ps = ppmm.tile([P, 512], F32, tag="mm", name="ps_qk")
                    for o in range(DO):
                        nc.tensor.matmul(
                            ps[:], wt[:, o, m * P:(m + 1) * P],
                            src[:, o, n * 512:(n + 1) * 512],
                            start=(o == 0), stop=(o == DO - 1))
                    mm_chain(nc.scalar.activation(
                        out[:, m, n * 512:(n + 1) * 512], ps[:],
                        AF.Identity, bias=bias_t[:, m:m + 1]))

        def v_proj(out, wt, bias_row, src, hl, seq):
            # out: [P(tok), seq//P, hl, 2*DK]; cols DK: stay for the ones blk
            mh = hl * DK
            for ms in range(seq // P):
                ps = ppmm.tile([P, 512], F32, tag="mm", name="ps_v")
                for o in range(DO):
                    nc.tensor.matmul(
                        ps[:, 0:mh], src[:, o, ms * P:(ms + 1) * P],
                        wt[:, o, :], start=(o == 0),
                        stop=(v_bias_zero and o == DO - 1))
                if not v_bias_zero:
                    nc.tensor.matmul(ps[:, 0:mh], onesb[0:1, :],
                                     bias_row[0:1, :], start=False, stop=True)
                mm_chain(nc.scalar.copy(
                    out[:, ms, :, 0:DK],
                    ps[:, 0:mh].rearrange("p (h d) -> p h d", h=hl)))

        # ---- self QKV projections ----------------------------------------
        # KT1z[z] is the K block for head z of the pair, zero-padded to the
        # full 128 partitions, so score matmuls run 128-contract like every
        # other matmul (no PE row-mode switches against the attV matmuls).
        QT1 = phB.tile([P, 1, TQ], BF16, name="QT1")
        KT1z = phB.tile([P, 2, TQ], BF16, name="KT1z")
        V1 = phB.tile([P, NKs, HLs, 2 * DK], BF16, name="V1")
        qk_proj(QT1, WQ1, bq1t, XTt, 1, TQ)
        nc.gpsimd.memset(KT1z[DK:2 * DK, 0, :], 0.0)
        nc.gpsimd.memset(KT1z[0:DK, 1, :], 0.0)
        for n in range(TQ // 512):
            ps = ppmm.tile([P, 512], F32, tag="mm", name="ps_k1")
            for o in range(DO):
                nc.tensor.matmul(
                    ps[:], WK1[:, o, :], XTt[:, o, n * 512:(n + 1) * 512],
                    start=(o == 0), stop=(o == DO - 1))
            mm_chain(nc.scalar.activation(
                KT1z[0:DK, 0, n * 512:(n + 1) * 512], ps[0:DK],
                AF.Identity, bias=bk1t[0:DK, 0:1]))
            mm_chain(nc.scalar.activation(
                KT1z[DK:2 * DK, 1, n * 512:(n + 1) * 512], ps[DK:2 * DK],
                AF.Identity, bias=bk1t[DK:2 * DK, 0:1]))
        v_proj(V1, WV1, bv1t, XTt, HLs, TQ)
        nc.gpsimd.memset(V1[:, :, :, DK:2 * DK], 1.0)
        pio_x_cm.__exit__(None, None, None)

        # ---- attention helper --------------------------------------------
        def attention_pair(qt_m, KT, QT, Vt, he, ho, qb, qw, cls_row,
                           strips, Xatt, xm, nk):
            """One (head-pair, q-block).  Even head lives on SBUF partitions
            0-63, odd head on 64-127.  Vt[..., DK:2DK] is an all-ones block,
            so each attV matmul emits 64 broadcast copies of the softmax
            denominator on partitions 64-127; normalization is pure DVE.
            Chunks are software-pipelined: scores/exp of chunk c+1 are issued
            before the attV matmuls of chunk c so the PE never waits on the
            scalar engine's exp."""
            allowed = [kb for kb in range(nk) if cls_row[kb] != SKIP]
            n = len(allowed)
            pa_e = ppatt.tile([P, 512], F32, tag="att", name="pa_e")
            pa_o = ppatt.tile([P, 512], F32, tag="att", name="pa_o")

            def do_scores(chunk, after=None):
                E = patt.tile([P, 2, ECH, 512], BF16, tag="E", name="E")
                mms = []
                for j, kb in enumerate(chunk):
                    for half in (0, 1):
                        ps = ppmm.tile([P, 512], F32, tag="mm", name="ps_s")
                        mm = nc.tensor.matmul(
                            ps[:, 0:qw],
                            KT[:, half, kb * P:(kb + 1) * P],
                            QT[:, qt_m, qb * qw:(qb + 1) * qw],
                            start=True, stop=True)
                        mms.append(mm)
                        if cls_row[kb] >= 0:
                            nc.vector.tensor_tensor(
                                ps[:, 0:qw], ps[:, 0:qw],
                                strips[:, cls_row[kb], 0:qw], ALU.add)
                        mm_chain(nc.scalar.activation(
                            E[:, half, j, 0:qw], ps[:, 0:qw],
                            AF.Exp, bias=shiftc[:, 0:1]))
                return E, mms[-1]

            def do_attv(E, chunk, c0, after=None):
                last = None
                for j, kb in enumerate(chunk):
                    mm = nc.tensor.matmul(pa_e[:, 0:qw], Vt[:, kb, he, :],
                                          E[:, 0, j, 0:qw],
                                          start=(c0 + j == 0),
                                          stop=(c0 + j == n - 1))
                    last = nc.tensor.matmul(pa_o[:, 0:qw], Vt[:, kb, ho, :],
                                            E[:, 1, j, 0:qw],
                                            start=(c0 + j == 0),
                                            stop=(c0 + j == n - 1))
                return last

            chunks = [allowed[c0:c0 + ECH] for c0 in range(0, n, ECH)]
            prevE, lastS = do_scores(chunks[0])
            prevA = None
            prev0 = 0
            for ci in range(1, len(chunks)):
                E2, lastS2 = do_scores(chunks[ci], after=prevA)
                prevA = do_attv(prevE, chunks[ci - 1], prev0, after=lastS2)
                prevE, lastS = E2, lastS2
                prev0 += len(chunks[ci - 1])
            do_attv(prevE, chunks[-1], prev0, after=None)

            # denominators -> bf16 SBUF (scalar), one fast-mode DVE recip
            den = psm.tile([P, 512], BF16, tag="den", name="den")
            nc.vector.tensor_copy(den[0:DK, 0:qw], pa_e[DK:2 * DK, 0:qw])
            nc.vector.tensor_copy(den[DK:2 * DK, 0:qw], pa_o[DK:2 * DK, 0:qw])
            rec = psm.tile([P, 512], BF16, tag="rec", name="rec")
            with nc.allow_low_precision(reason="softmax denom recip in bf16"):
                nc.vector.reciprocal(rec[:, 0:qw], den[:, 0:qw])
            nc.vector.tensor_tensor(
                Xatt[0:DK, xm, 0:qw],
                pa_e[0:DK, 0:qw], rec[0:DK, 0:qw], ALU.mult)
            nc.vector.tensor_tensor(
                Xatt[DK:2 * DK, xm, 0:qw],
                pa_o[0:DK, 0:qw], rec[DK:2 * DK, 0:qw], ALU.mult)

        # ---- self attention + AllToAll context exchange ------------------
        # cross K/V projection + its AllGather are emitted after qb=1 so the
        # enc_out load hides under early attention and the AG under the rest.
        HB = SB // 2
        a2a_inA = dram.tile([cfg.n_cores, P, HB], BF16, name="a2a_inA")
        a2a_inB = dram.tile([cfg.n_cores, P, HB], BF16, name="a2a_inB")

        def self_qb(qb):
            xst = pln.tile([P, 1, SB], BF16, tag="xst", name="xst", bufs=2)
            attention_pair(0, KT1z, QT1, V1, 0, 1, qb, 512, self_cls[qb],
                           strSt, xst, 0, NKs)
            nc.sync.dma_start(a2a_inA[qb], xst[:, 0, 0:HB])
            nc.sync.dma_start(a2a_inB[qb], xst[:, 0, HB:SB])

        sz = lambda q: sum(1 for v in self_cls[q] if v != SKIP)
        by_size = sorted(range(NQ), key=sz)
        qb_order = by_size[:2] + by_size[:1:-1]
        self_qb(qb_order[0])
        self_qb(qb_order[1])
        # ---- cross K/V projections + AllGather (hides under self-attn) ---
        pA_kv_cm = tc.tile_pool(name="pA_kv", bufs=1)
        pA_kv = pA_kv_cm.__enter__()
        pio_e_cm = tc.tile_pool(name="pio_e", bufs=1)
        pio_e = pio_e_cm.__enter__()
        ENCTt = pio_e.tile([P, DO, S], BF16, name="ENCTt")
        for o in range(DO):
            nc.sync.dma_start(ENCTt[:, o, :], encT[:, o, :])
        WK2 = pio_e.tile([P, DO, MHc], BF16, name="WK2")
        nc.sync.dma_start(WK2[:], wk2)
        WV2 = pio_e.tile([P, DO, MHc], BF16, name="WV2")
        nc.sync.dma_start(WV2[:], wv2)
        KT2l = pA_kv.tile([P, MOc, S], BF16, name="KT2l")
        V2l = pA_kv.tile([P, NKc, HLc, DK], BF16, name="V2l")
        qk_proj(KT2l, WK2, bk2t, ENCTt, MOc, S)
        for ms in range(S // P):
            ps = ppmm.tile([P, 512], F32, tag="mm", name="ps_v2")
            for o in range(DO):
                nc.tensor.matmul(
                    ps[:, 0:MHc], ENCTt[:, o, ms * P:(ms + 1) * P],
                    WV2[:, o, :], start=(o == 0),
                    stop=(v_bias_zero and o == DO - 1))
            if not v_bias_zero:
                nc.tensor.matmul(ps[:, 0:MHc], onesb[0:1, :],
                                 bv2t[0:1, :], start=False, stop=True)
            mm_chain(nc.scalar.copy(
                V2l[:, ms, :, :],
                ps[:, 0:MHc].rearrange("p (h d) -> p h d", h=HLc)))
        pio_e_cm.__exit__(None, None, None)

        KSZ = MOc * S
        VSZ = NKc * HLc * DK
        kv_loc = dram.tile([P, KSZ + VSZ], BF16, name="kv_loc")
        nc.sync.dma_start(kv_loc[:, 0:KSZ],
                          KT2l[:].rearrange("p m s -> p (m s)"))
        nc.sync.dma_start(kv_loc[:, KSZ:KSZ + VSZ],
                          V2l[:].rearrange("p k h d -> p (k h d)"))
        kv_ag = dram.tile([G, P, KSZ + VSZ], BF16, name="kv_ag")
        nc.gpsimd.collective_compute(
            "AllGather", ALU.bypass, replica_groups=groups4,
            ins=[kv_loc.opt()], outs=[kv_ag.opt()])
        pA_kv_cm.__exit__(None, None, None)

        # post-attention working set (region reuse; DMAs run under attn) ---
        pO1_cm = tc.tile_pool(name="pO1", bufs=1)
        pO1 = pO1_cm.__enter__()
        xckt = pO1.tile([P, DO, SB], F32, name="xckt")
        nc.sync.dma_start(xckt[:], xck)
        WO1t = pO1.tile([P, DO, D], BF16, name="WO1t")
        nc.sync.dma_start(WO1t[:], wo1f)

        for qb in qb_order[2:]:
            self_qb(qb)
        a2a_outA = dram.tile([cfg.n_cores, P, HB], BF16, name="a2a_outA")
        a2a_outB = dram.tile([cfg.n_cores, P, HB], BF16, name="a2a_outB")
        nc.gpsimd.collective_compute(
            "AllToAll", ALU.bypass, replica_groups=groups8,
            ins=[a2a_inA.opt()], outs=[a2a_outA.opt()])
        nc.gpsimd.collective_compute(
            "AllToAll", ALU.bypass, replica_groups=groups8,
            ins=[a2a_inB.opt()], outs=[a2a_outB.opt()])
        XA = pO1.tile([P, DO, SB], BF16, name="XA")
        for j in range(cfg.n_cores):
            nc.sync.dma_start(XA[:, j, 0:HB], a2a_outA[j])
        for j in range(cfg.n_cores):
            nc.sync.dma_start(XA[:, j, HB:SB], a2a_outB[j])

        # ---- LN helpers (stats pre-broadcast via all-ones stationary;
        # per-m stat matmuls are emitted lag-one inside the producer loops
        # so the PE never drains between a projection and its LN) ----------
        def ln_stats_begin():
            psA = ppatt.tile([P, 512], F32, tag="att", name="psA")
            psB = ppatt.tile([P, 512], F32, tag="att", name="psB")
            return psA, psB

        def ln_stat_m(st, S_sb, m, W):
            psA, psB = st
            sbf = pln.tile([P, W], BF16, tag="sbf", name="sbf", bufs=2)
            sq = pln.tile([P, W], BF16, tag="sq", name="sq", bufs=2)
            nc.vector.tensor_copy(sbf[:], S_sb[:, m])
            nc.vector.tensor_mul(sq[:], S_sb[:, m], S_sb[:, m])
            nc.tensor.matmul(psA[:, 0:W], ones128[:], sbf[:],
                             start=(m == 0), stop=(m == DO - 1))
            nc.tensor.matmul(psB[:, 0:W], ones128[:], sq[:],
                             start=(m == 0), stop=(m == DO - 1))

        def ln_finish(st, S_sb, gt, ct, out_f32, out_bf16, W):
            psA, psB = st
            mu = psm.tile([P, 512], F32, tag="stat", name="mu", bufs=4)
            nc.scalar.activation(mu[:, 0:W], psA[:, 0:W], AF.Copy,
                                 scale=1.0 / D)
            e2 = psm.tile([P, 512], F32, tag="stat", name="e2", bufs=4)
            nc.scalar.activation(e2[:, 0:W], psB[:, 0:W], AF.Copy,
                                 scale=1.0 / D)
            musq = psm.tile([P, 512], F32, tag="stat", name="musq", bufs=4)
            nc.vector.tensor_mul(musq[:, 0:W], mu[:, 0:W], mu[:, 0:W])
            nc.vector.tensor_sub(musq[:, 0:W], musq[:, 0:W], e2[:, 0:W])
            # musq now holds -var; sd = sqrt(-musq + eps); rstd = 1/sd
            sd = psm.tile([P, 512], BF16, tag="sdb", name="sd", bufs=2)
            nc.scalar.activation(sd[:, 0:W], musq[:, 0:W], AF.Sqrt,
                                 bias=epsc[:, 0:1], scale=-1.0)
            rstd = psm.tile([P, 512], BF16, tag="sdb", name="rstd", bufs=2)
            with nc.allow_low_precision(reason="LN rstd in bf16"):
                nc.vector.reciprocal(rstd[:, 0:W], sd[:, 0:W])
            mr = psm.tile([P, 512], F32, tag="stat", name="mr", bufs=4)
            nc.vector.tensor_mul(mr[:, 0:W], mu[:, 0:W], rstd[:, 0:W])
            for m in range(DO):
                t2 = pln.tile([P, W], F32, tag="t2", name="t2", bufs=2)
                nc.vector.tensor_mul(t2[:], S_sb[:, m], rstd[:, 0:W])
                nc.vector.tensor_sub(t2[:], t2[:], mr[:, 0:W])
                nc.scalar.activation(out_f32[:, m], t2[:], AF.Identity,
                                     bias=ct[:, m:m + 1], scale=gt[:, m:m + 1])
                if out_bf16 is not None:
                    nc.vector.tensor_copy(out_bf16[:, m], out_f32[:, m])

        # ---- O1 projection (full D) + residual + LN1 ---------------------
        S1 = pO1.tile([P, DO, SB], F32, name="S1")
        st1 = ln_stats_begin()
        for h in range(2):
            cr = slice(h * HB, (h + 1) * HB)
            for m in range(DO):
                ps = ppmm.tile([P, 512], F32, tag="mm", name="ps_o1")
                for j in range(DO):
                    nc.tensor.matmul(
                        ps[:, 0:HB], WO1t[:, j, m * P:(m + 1) * P],
                        XA[:, j, cr], start=(j == 0), stop=(j == DO - 1))
                mm_chain(nc.vector.scalar_tensor_tensor(
                    S1[:, m, cr], ps[:, 0:HB], bo1t[:, m:m + 1],
                    xckt[:, m, cr], ALU.add, ALU.add))
                if h == 1 and m > 0:
                    ln_stat_m(st1, S1, m - 1, SB)
        ln_stat_m(st1, S1, DO - 1, SB)
        X2f = pC.tile([P, DO, SB], F32, name="X2f")
        X2b = pC.tile([P, DO, SB], BF16, name="X2b")
        ln_finish(st1, S1, g1t, c1t, X2f, X2b, SB)
        pO1_cm.__exit__(None, None, None)
        phB_cm.__exit__(None, None, None)

        # cross-attn K/V staging enters now: its DMAs overlap LN1/Q2proj
        pva_cm = tc.tile_pool(name="pva", bufs=1)
        pva = pva_cm.__enter__()
        pkt_cm = tc.tile_pool(name="pkt", bufs=1)
        pkt = pkt_cm.__enter__()

        # ---- cross-attn Q projection (token-sharded, all heads) ----------
        Q2T = pC.tile([P, DO, SB], BF16, name="Q2T")
        for m in range(DO):
            wq2t = pw.tile([P, DO, P], BF16, tag="wsm", name="wq2t")
            nc.sync.dma_start(wq2t[:], wq2p[:, m])
            ps = ppmm.tile([P, 512], F32, tag="mm", name="ps_q2")
            for o in range(DO):
                nc.tensor.matmul(ps[:, 0:SB], wq2t[:, o, :], X2b[:, o, :],
                                 start=(o == 0), stop=(o == DO - 1))
            mm_chain(nc.scalar.activation(Q2T[:, m, :], ps[:, 0:SB],
                                          AF.Identity, bias=bq2t[:, m:m + 1]))

        # ---- cross attention (two head-half passes, double-buffered) -----
        Xatt2 = pC.tile([P, DO, SB], BF16, name="Xatt2")
        cross_cls = [kb if nsc > 0 else FULL for kb in range(NKc)]
        HH = H // 2
        ktpz = [pkt.tile([P, 2, S], BF16, name=f"ktpz{i}") for i in range(2)]
        for i in range(2):
            nc.gpsimd.memset(ktpz[i][DK:2 * DK, 0, :], 0.0)
            nc.gpsimd.memset(ktpz[i][0:DK, 1, :], 0.0)
        for half in range(2):
            Va = pva.tile([P, NKc, HH, 2 * DK], BF16, tag="va", name="Va",
                          bufs=2)
            h0 = half * HH
            for r in range(G):
                lo = max(h0, r * HLc)
                hi = min(h0 + HH, (r + 1) * HLc)
                if lo >= hi:
                    continue
                for h in range(lo, hi):
                    nc.sync.dma_start(
                        Va[:, :, h - h0, 0:DK],
                        kv_ag[r, :, KSZ:KSZ + VSZ].rearrange(
                            "p (k h d) -> p k h d", k=NKc, h=HLc)
                        [:, :, h - r * HLc, :])
            nc.gpsimd.memset(Va[:, :, :, DK:2 * DK], 1.0)
            for hp in range(h0 // 2, (h0 + HH) // 2):
                r, mo = divmod(hp, MOc)
                kz = ktpz[hp % 2]
                nc.sync.dma_start(kz[0:DK, 0, :],
                                  kv_ag[r, 0:DK, mo * S:(mo + 1) * S])
                nc.sync.dma_start(kz[DK:2 * DK, 1, :],
                                  kv_ag[r, DK:2 * DK, mo * S:(mo + 1) * S])
                attention_pair(hp, kz, Q2T, Va, 2 * hp - h0,
                               2 * hp + 1 - h0, 0, SB, cross_cls, strCt,
                               Xatt2, hp, NKc)
        pkt_cm.__exit__(None, None, None)
        pva_cm.__exit__(None, None, None)
        pff_cm = tc.tile_pool(name="pff", bufs=1)
        pff = pff_cm.__enter__()

        # ---- cross O-projection + residual (in-place over X2f) + LN2 ----
        st2 = ln_stats_begin()
        for m in range(DO):
            wo2t = pw.tile([P, DO, P], BF16, tag="wsm", name="wo2t")
            nc.sync.dma_start(wo2t[:], wo2p[:, m])
            ps = ppmm.tile([P, 512], F32, tag="mm", name="ps_o2")
            for o in range(DO):
                nc.tensor.matmul(ps[:, 0:SB], wo2t[:, o, :], Xatt2[:, o, :],
                                 start=(o == 0), stop=(o == DO - 1))
            mm_chain(nc.vector.scalar_tensor_tensor(
                X2f[:, m], ps[:, 0:SB], bo2t[:, m:m + 1], X2f[:, m],
                ALU.add, ALU.add))
            if m > 0:
                ln_stat_m(st2, X2f, m - 1, SB)
        ln_stat_m(st2, X2f, DO - 1, SB)
        X4f = pff.tile([P, DO, SB], F32, name="X4f")
        X4b = pff.tile([P, DO, SB], BF16, name="X4b")
        ln_finish(st2, X2f, g2t, c2t, X4f, X4b, SB)

        # ---- FFN (two FO-halves; accumulate into X4f in place) ----------
        st3 = ln_stats_begin()
        FH = FO // 2
        for half in range(2):
            f0 = half * FH
            Ht = pff.tile([P, FH, SB], BF16, tag="Ht", name="Ht")
            for mf in range(FH):
                w1t = pw.tile([P, DO, P], BF16, tag="wsm", name="w1t")
                nc.sync.dma_start(w1t[:], w1p[:, f0 + mf])
                ps = ppmm.tile([P, 512], F32, tag="mm", name="ps_f1")
                for o in range(DO):
                    nc.tensor.matmul(ps[:, 0:SB], w1t[:, o, :], X4b[:, o, :],
                                     start=(o == 0), stop=(o == DO - 1))
                mm_chain(nc.scalar.activation(
                    Ht[:, mf, :], ps[:, 0:SB], AF.Relu,
                    bias=b1t[:, f0 + mf:f0 + mf + 1]))
            for m in range(DO):
                w2t = pw.tile([P, FH, P], BF16, tag="w2", name="w2t")
                nc.sync.dma_start(w2t[:], w2p[:, m, f0:f0 + FH, :])
                ps = ppmm.tile([P, 512], F32, tag="mm", name="ps_f2")
                for of in range(FH):
                    nc.tensor.matmul(ps[:, 0:SB], w2t[:, of, :], Ht[:, of, :],
                                     start=(of == 0), stop=(of == FH - 1))
                if half == 0:
                    mm_chain(nc.vector.scalar_tensor_tensor(
                        X4f[:, m], ps[:, 0:SB], b2t[:, m:m + 1], X4f[:, m],
                        ALU.add, ALU.add))
                else:
                    mm_chain(nc.vector.tensor_add(
                        X4f[:, m], X4f[:, m], ps[:, 0:SB]))
                    if m > 0:
                        ln_stat_m(st3, X4f, m - 1, SB)
        ln_stat_m(st3, X4f, DO - 1, SB)
        ln_finish(st3, X4f, g3t, c3t, X4f, None, SB)
        for m in range(DO):
            nc.sync.dma_start(y[m], X4f[:, m])

        pff_cm.__exit__(None, None, None)
        patt_cm.__exit__(None, None, None)
        pln_cm.__exit__(None, None, None)
        pw_cm.__exit__(None, None, None)
        pC_cm.__exit__(None, None, None)
        psm_cm.__exit__(None, None, None)
        ppatt_cm.__exit__(None, None, None)
        ppmm_cm.__exit__(None, None, None)
        dram_cm.__exit__(None, None, None)
        res_cm.__exit__(None, None, None)

    nc.compile()
    return nc


# ---------------------------------------------------------------------------
# host side
# ---------------------------------------------------------------------------

def _pack_ko(a):
    """[K, F] -> [128, K//128, F] (contract dim on partitions)."""
    K, F = a.shape
    return np.ascontiguousarray(a.reshape(K // P, P, F).transpose(1, 0, 2))


def _pack_vec(v, n):
    return np.ascontiguousarray(np.asarray(v, np.float32).reshape(n, P).T)


def classify_self(mask, NQ, NK):
    """mask [S, S] bool (q, k). Returns cls [NQ][NK] and strips [128, nss, 512]."""
    cls = [[FULL] * NK for _ in range(NQ)]
    strips = []
    keys = {}
    for qb in range(NQ):
        for kb in range(NK):
            blk = mask[qb * 512:(qb + 1) * 512, kb * P:(kb + 1) * P]
            if blk.all():
                cls[qb][kb] = FULL
            elif not blk.any():
                cls[qb][kb] = SKIP
            else:
                key = blk.tobytes()
                if key not in keys:
                    keys[key] = len(strips)
                    strips.append(np.where(blk.T, np.float32(0),
                                           np.float32(NEG)))
                cls[qb][kb] = keys[key]
    if strips:
        arr = np.stack(strips, 0).transpose(1, 0, 2)
    else:
        arr = np.zeros((P, 1, 512), np.float32)
    return cls, np.ascontiguousarray(arr).astype(bfloat16)


_CACHE = {}


def kernel(**inputs):
    cfg = Cfg(S=2048, D=1024, H=16, FF=4096, TP=4, B=2)
    return _run(cfg, inputs)


def _run(cfg, inputs, trace=False):
    S, D, G, B, SB, DO = cfg.S, cfg.D, cfg.G, cfg.B, cfg.SB, cfg.DO
    MHs, MHc, MOc, NQ, NKs, NKc = (cfg.MHs, cfg.MHc, cfg.MOc, cfg.NQ,
                                   cfg.NKs, cfg.NKc)
    f32 = np.float32
    bf = bfloat16
    tgt = np.asarray(inputs["tgt_mask"])[0, 0] != 0
    src = np.asarray(inputs["src_mask"])[0, 0] != 0

    # per-batch causal classification, composed block-diagonally over B
    clsb, strS = classify_self(tgt, S // 512, S // P)
    nqb, nkb = S // 512, S // P
    self_cls = [[SKIP] * NKs for _ in range(NQ)]
    for qb in range(NQ):
        for kb in range(NKs):
            if qb // nqb == kb // nkb:
                self_cls[qb][kb] = clsb[qb % nqb][kb % nkb]
    nsc = 0 if src.all() else NKc

    v_bias_zero = (not np.asarray(inputs["m1_bv"]).any()) and \
        (not np.asarray(inputs["m2_bv"]).any())
    key = (cfg.S, cfg.D, cfg.H, cfg.FF, cfg.G, cfg.B,
           tuple(map(tuple, self_cls)), nsc, v_bias_zero)
    if key not in _CACHE:
        _CACHE[key] = build_program(cfg, self_cls, nsc, v_bias_zero)
    nc = _CACHE[key]

    x = np.asarray(inputs["x"], f32)
    enc = np.asarray(inputs["enc_out"], f32)
    w1 = np.asarray(inputs["ff_w1"], f32)
    w2 = np.asarray(inputs["ff_w2"], f32)
    wq2 = np.asarray(inputs["m2_wq"], f32)
    wo2 = np.asarray(inputs["m2_wo"], f32)

    # xT: both batches concatenated on the token axis (batch-major)
    xT_full = np.concatenate([x[0], x[1]], axis=0).T  # [D, TQ]
    xT_pack = np.ascontiguousarray(
        xT_full.reshape(DO, P, cfg.TQ).transpose(1, 0, 2)).astype(bf)

    shared = {
        "xT": xT_pack,
        "wo1f": _pack_ko(np.asarray(inputs["m1_wo"], f32)).astype(bf),
        "wq2p": np.ascontiguousarray(
            wq2.reshape(DO, P, DO, P).transpose(1, 2, 0, 3)).astype(bf),
        "bq2": _pack_vec(inputs["m2_bq"], DO),
        "wo2p": np.ascontiguousarray(
            wo2.reshape(DO, P, DO, P).transpose(1, 2, 0, 3)).astype(bf),
        "bo2": _pack_vec(inputs["m2_bo"], DO),
        "bo1": _pack_vec(inputs["m1_bo"], DO),
        "w1p": np.ascontiguousarray(
            w1.reshape(DO, P, cfg.FO, P).transpose(1, 2, 0, 3)).astype(bf),
        "b1": _pack_vec(inputs["ff_b1"], cfg.FO),
        "w2p": np.ascontiguousarray(
            w2.reshape(cfg.FO, P, DO, P).transpose(1, 2, 0, 3)).astype(bf),
        "b2": _pack_vec(inputs["ff_b2"], DO),
        "g1": _pack_vec(inputs["ln1_g"], DO),
        "c1": _pack_vec(inputs["ln1_b"], DO),
        "g2": _pack_vec(inputs["ln2_g"], DO),
        "c2": _pack_vec(inputs["ln2_b"], DO),
        "g3": _pack_vec(inputs["ln3_g"], DO),
        "c3": _pack_vec(inputs["ln3_b"], DO),
        "strS": strS,
    }

    in_maps = []
    for c in range(cfg.n_cores):
        b, r = divmod(c, G)
        xTb = np.ascontiguousarray(x[b].T)
        encTb = np.ascontiguousarray(enc[b].T)
        m = dict(shared)
        m["xck"] = np.ascontiguousarray(
            xTb[:, r * SB:(r + 1) * SB].reshape(DO, P, SB).transpose(1, 0, 2))
        m["encT"] = np.ascontiguousarray(
            encTb.reshape(DO, P, S).transpose(1, 0, 2)).astype(bf)
        # self-attn: 2 heads per core (TP=8 over heads)
        hs = slice(c * MHs, (c + 1) * MHs)
        m["wq1"] = _pack_ko(np.asarray(inputs["m1_wq"], f32)[:, hs]).astype(bf)
        m["wk1"] = _pack_ko(np.asarray(inputs["m1_wk"], f32)[:, hs]).astype(bf)
        m["wv1"] = _pack_ko(np.asarray(inputs["m1_wv"], f32)[:, hs]).astype(bf)
        m["bq1"] = _pack_vec(np.asarray(inputs["m1_bq"], f32)[hs], 1)
        m["bk1"] = _pack_vec(np.asarray(inputs["m1_bk"], f32)[hs], 1)
        m["bv1"] = np.asarray(inputs["m1_bv"], f32)[hs].reshape(1, MHs).astype(bf)
        # cross-attn K/V: 4 heads per group rank
        hc = slice(r * MHc, (r + 1) * MHc)
        m["wk2"] = _pack_ko(np.asarray(inputs["m2_wk"], f32)[:, hc]).astype(bf)
        m["wv2"] = _pack_ko(np.asarray(inputs["m2_wv"], f32)[:, hc]).astype(bf)
        m["bk2"] = _pack_vec(np.asarray(inputs["m2_bk"], f32)[hc], MOc)
        m["bv2"] = np.asarray(inputs["m2_bv"], f32)[hc].reshape(1, MHc).astype(bf)
        if nsc > 0:
            blks = []
            for kb in range(NKc):
                blk = src[r * SB:(r + 1) * SB, kb * P:(kb + 1) * P]
                blks.append(np.where(blk.T, f32(0), f32(NEG)))
            m["strC"] = np.ascontiguousarray(
                np.stack(blks, 0).transpose(1, 0, 2)).astype(bf)
        else:
            m["strC"] = np.zeros((P, 1, SB), bf)
        in_maps.append(m)

    res = bass_utils.run_bass_kernel_spmd(
        nc, in_maps, core_ids=list(range(cfg.n_cores)), trace=trace)

    out = np.empty((B, S, D), f32)
    for c in range(cfg.n_cores):
        b, r = divmod(c, G)
        yv = res.results[c]["y"]
        out[b, r * SB:(r + 1) * SB, :] = yv.transpose(2, 0, 1).reshape(SB, D)
    if trace:
        return out, res
    return out
